# revision 32
# baseline (speedup 1.0000x reference)
"""Trainium2 Bass kernel for nn_Attention_42657615184259.

Multi-head attention block: x:[8,2048,384] -> qkv proj -> 6-head SDPA
(full softmax) -> out proj -> y:[8,2048,384].

Sharding: data-parallel over batch B=8, one batch element per NeuronCore.

v2: the 25M-exp softmax is split across TWO engines: ScalarE keeps most
chunks (exp table, scale=ln2/128; q pre-scaled by 128*SCALE*log2e on
host) and 5-7 of 16 key-chunks per (head, half) run on the Vector
engine via EXP2_BITS_ANT, a registered custom 8-slice DVE op computing
bf16 exp2 bits with a magic-constant floor + quadratic minimax fit,
written as int16 (max rel err 0.64%). The exponent bias 127*128 rides
the score matmul as a 65th contraction row (16192 in q65/k65 side
tiles), so the op needs no extra constant slot. Softmax normalization
cancels the op's uniform gain. Norm multiplies and bias-row memsets run
on GpSimd; proj contracts head PAIRS (128-deep) against row-stacked
bf16 proj weights; AV emits kc-outer so both q-blocks share one
LDWEIGHTS; a dummy exp at t~0 preloads the ACT table set under the x
DMAs. The DVE exp chunks are spread through each half (dve_set, with
packed per-head k65 slot tiles) so both exp engines run concurrently
from the first kc instead of DVE bunching at the tail. Measured
284.6us/core cold (the device P0-throttles ~1.2x under sustained
back-to-back runs; baseline was 291-344), rel err 5.1e-3.

Per-core design (everything in "transposed" space, contraction dims on
SBUF partitions; all matmuls bf16, accumulation/psum fp32):
  1. xT[c,n] built from x (host-cast to bf16) via PE transposes.
  2. qkT[j,n] = qkv_w[:768] @ x.T, stored bf16. v kept in natural layout
     v'[n, h, 0:64] with a ones column at [:, h, 64] so the AV matmul's
     extra output row yields the softmax denominators Z for free.
  3. Per head: scoresT[k,q] = kT.T @ qT -> exp(SCALE*s) on ScalarE
     (PSUM->SBUF, bf16) -> out'[0:65, q] += v'_h.T @ expT over k chunks.
  4. 1/Z per half, reshaped to [128, n/128] for the reciprocal, bounced
     through DRAM with a step-0 partition AP to broadcast across
     partitions, then attnT normalized in place.
  5. proj accumulates all heads in PSUM; + bias on DVE; DMA out.
The emission order interleaves qkT/v'/x-transposes under head-0/1/2's
exp stream so ScalarE (the 25M-exp bottleneck, ~214us busy) saturates
from ~19us on: dependency-free warm-up matmuls lift the PE HAM clock
gate at t~0 and the first four score chunks use 512-wide bites so exp
starts after the first quarter of x. ScalarE and the PE are both
~95-100% busy during the main phase. Best measured 291us/core on TRN2
(run-to-run machine variance ~291-330us).
"""

import os
import numpy as np
from contextlib import ExitStack

DIM = 384
HEADS = 6
DK = 64
N_TOK = 2048
B = 8
N_CORES = 8

_module_cache = {}

# ---- EXP2_BITS_ANT: one-instruction exp2-to-bf16-bits on the DVE ----
# Z = ((x*C2 + C1)*x) + (Src0 + Src1); T = Src0 + C0; R = T - C0; x = Src0 - R
# Src0 = U = 128*u + 16192 (bias via extra score-matmul contraction row);
# int16 write = bf16 bits of 2^u * (1 + ~2e-4). Max rel err 0.64%, rms 0.21%.
EXP2_MAGIC = 1.5 * 2**30
EXP2_C_LIN = -0.0049568056
EXP2_C_QUAD = 0.0026875064
EXP2_C_CONST = 52.991974
EXP2_B_ROW = 16192.0
EXP2_ACT_SCALE = 0.6931471805599453 / 128.0  # ln(2)/128


def _register_exp2():
    from concourse import dve_ops as D
    from concourse.dve_spec import Spec, Src0, Src1, C0, C1, C2, lower, _has_src1
    from concourse.dve_uop import DveOpSpec
    import numpy as _np

    name = "EXP2_BITS_ANT"
    for op in D.OPS:
        if op.name == name:
            return op
    T = Src0 + C0
    R = T - C0
    x = Src0 - R
    Z = ((x * C2 + C1) * x) + (Src0 + Src1)

    def ref(in0, in1, s0, s1, imm2):
        f32 = _np.float32
        in0 = in0.astype(f32)
        t = (in0 + f32(s0)).astype(f32)
        r = (t - f32(s0)).astype(f32)
        xx = (in0 - r).astype(f32)
        p = (((xx * f32(imm2)).astype(f32) + f32(s1)).astype(f32) * xx).astype(f32)
        return p + (in0 + in1.astype(f32)).astype(f32)

    spec = Spec(body=Z, reference=ref)
    row = D._CUSTOM_DVE_ROW_BASE + len(D.OPS)
    assert row < 0x20
    D._SUB_OPCODE_FOR_NAME[name] = row
    shas = {}
    for ver in ("v3", "v4"):
        uops = lower(spec, ver=ver)
        s = DveOpSpec(name=name, opcode=row, uops=uops, rd1_en=_has_src1(spec))
        shas[ver] = s.sha(ver)
    op = D.DveOp(name, spec, subdim=False, uops_sha=shas)
    D.OPS.append(op)
    D.CUSTOM_DVE_SPECS[name] = spec
    return op


def build_module(n_tok=N_TOK, dim=DIM, heads=HEADS, debug=False):
    """Build + compile the per-core Bass module. Returns the Bacc object."""
    import concourse.bass as bass
    import concourse.tile as tile
    from concourse import bacc, mybir
    from concourse.masks import make_identity

    f32 = mybir.dt.float32
    bf16 = mybir.dt.bfloat16
    i16 = mybir.dt.int16
    AF = mybir.ActivationFunctionType
    ALU = mybir.AluOpType
    EXP2 = _register_exp2()

    assert dim % 128 == 0 and n_tok % 2048 == 0 and dim == heads * DK
    CC = dim // 128          # contraction chunks over model dim
    JC = 2 * dim // 128      # q+k row chunks
    NCH = n_tok // 128       # token chunks of 128
    NQ4 = n_tok // 512       # token chunks of 512
    HALF = n_tok // 2
    SCALE = DK ** -0.5

    nc = bacc.Bacc("TRN2", target_bir_lowering=False, debug=debug)

    x_d = nc.dram_tensor("x_b", [n_tok, dim], bf16, kind="ExternalInput").ap()
    qkw_d = nc.dram_tensor("qkw_t", [dim, 2 * dim], bf16, kind="ExternalInput").ap()
    vw_d = nc.dram_tensor("vw_t", [dim, dim], bf16, kind="ExternalInput").ap()
    pw_d = nc.dram_tensor("pw_t", [dim, dim], bf16, kind="ExternalInput").ap()
    qkb_d = nc.dram_tensor("qk_b", [2 * dim], f32, kind="ExternalInput").ap()
    vb_d = nc.dram_tensor("v_b", [dim], f32, kind="ExternalInput").ap()
    pb_d = nc.dram_tensor("p_b", [dim], f32, kind="ExternalInput").ap()
    y_d = nc.dram_tensor("y_b", [n_tok, dim], f32, kind="ExternalOutput").ap()

    with tile.TileContext(nc) as tc, ExitStack() as es:
        consts = es.enter_context(tc.tile_pool(name="consts", bufs=1))
        persist = es.enter_context(tc.tile_pool(name="persist", bufs=1))

        # ---- constants / weights (on the gpsimd queue; sync is kept for x) ----
        # PE pre-warm: dependency-free matmuls starting at t~0 lift the HAM
        # clock gate to 2.4GHz before the first real transpose arrives
        junk_sb = consts.tile([128, 128], bf16, tag="junk", name="junk_sb")
        nc.vector.memset(junk_sb, 1.0)
        junk_e = consts.tile([128, 8], bf16, tag="junke", name="junk_e")
        ident = consts.tile([128, 128], bf16, tag="ident", name="ident")
        make_identity(nc, ident)
        qkwT = []
        vwT = []
        for cc in range(CC):
            t = consts.tile([128, 2 * dim], bf16, tag=f"qkw{cc}", name=f"qkw{cc}")
            nc.gpsimd.dma_start(out=t, in_=qkw_d[cc * 128:(cc + 1) * 128, :])
            qkwT.append(t)
            t = consts.tile([128, dim], bf16, tag=f"vw{cc}", name=f"vw{cc}")
            nc.gpsimd.dma_start(out=t, in_=vw_d[cc * 128:(cc + 1) * 128, :])
            vwT.append(t)
        qkb = []
        for jc in range(JC):
            t = consts.tile([128, 1], f32, tag=f"qkb{jc}", name=f"qkb{jc}")
            nc.gpsimd.dma_start(out=t, in_=qkb_d[jc * 128:(jc + 1) * 128])
            qkb.append(t)
        pwT = []
        for j in range(heads // 2):
            t = consts.tile([128, dim], bf16, tag=f"pw{j}", name=f"pw{j}")
            nc.gpsimd.dma_start(out=t, in_=pw_d[j * 128:(j + 1) * 128, :])
            pwT.append(t)
        cft = consts.tile([128, 1024], f32, tag="cft", name="cft")
        nc.vector.memset(cft, EXP2_C_CONST)
        # free-axis biases broadcast across partitions via step-0 DMA
        vb_bc = consts.tile([128, dim], f32, tag="vb", name="vb")
        nc.gpsimd.dma_start(
            out=vb_bc,
            in_=bass.AP(tensor=vb_d.tensor, offset=vb_d.offset,
                        ap=[[0, 128], *vb_d.ap]),
        )
        pb_bc = consts.tile([128, dim], f32, tag="pb", name="pb")
        nc.gpsimd.dma_start(
            out=pb_bc,
            in_=bass.AP(tensor=pb_d.tensor, offset=pb_d.offset,
                        ap=[[0, 128], *pb_d.ap]),
        )
        # persistent activations
        qkT = [persist.tile([128, n_tok], bf16, tag=f"qkT{jc}", name=f"qkT{jc}") for jc in range(JC)]
        vp = [persist.tile([128, heads, 65], bf16, tag=f"vp{ni}", name=f"vp{ni}") for ni in range(NCH)]

        atp = [persist.tile([128, n_tok], bf16, tag=f"atp{j}", name=f"atp{j}")
               for j in range(heads // 2)]
        attnT = [atp[h // 2][(h % 2) * 64:(h % 2) * 64 + 64, :] for h in range(heads)]
        # q65/k65: q,k slices + bias row (16192 / ones) for the 65-contraction
        # DVE-exp score matmuls (kc in DVE_KCS)
        # DVE-exp kc sets, spread through the half so both exp engines run
        # concurrently from the start (h0 keeps its tail-only set: half0 is
        # the ACT-saturated startup pipeline)
        dve_set = {0: {11, 12, 13, 14, 15},
                   1: {2, 4, 7, 9, 12, 14}, 2: {2, 4, 7, 9, 12, 14},
                   3: {2, 4, 6, 8, 10, 12, 14}, 4: {2, 4, 6, 8, 10, 12, 14},
                   5: {2, 4, 6, 8, 10, 12, 14}}
        q65 = [persist.tile([65, n_tok], bf16, tag=f"q65_{h}", name=f"q65_{h}")
               for h in range(heads)]
        k65 = [persist.tile([65, 7 * 128], bf16, tag=f"k65_{h}",
                            name=f"k65_{h}") for h in range(heads)]
        k65slot = {h: {kc: i for i, kc in enumerate(sorted(dve_set[h]))}
                   for h in range(heads)}
        for h in range(heads):
            nc.gpsimd.memset(q65[h][64:65, :], EXP2_B_ROW)
            nc.gpsimd.memset(k65[h][64:65, :], 1.0)

        def qk_slice(row0, col0, ncols):
            """[64, ncols] slice of the qkT row space at row row0 (64-aligned)."""
            ti, po = divmod(row0, 128)
            return qkT[ti][po:po + 64, col0:col0 + ncols]

        # ---- phases B+C, finely interleaved so ACT saturates early ----
        # PSUM budget at any emission point stays at 8 banks:
        #   sps(2x2) + avp(2x1) + bps(2x1, shared by x-transposes and qkT)
        projp = ypool = None

        def make_proj_pools():
            nonlocal projp, ypool
            if projp is None:
                projp = tc.alloc_tile_pool(name="projp", bufs=4, space="PSUM")
                ypool = tc.alloc_tile_pool(name="ypool", bufs=3)

        def drop_proj_pools():
            nonlocal projp, ypool
            if projp is not None:
                ypool.release()
                projp.release()
                projp = ypool = None

        def emit_proj_chunk(ni):
            yp = projp.tile([128, dim], f32, tag="yp", name="yp")
            for j in range(heads // 2):
                nc.tensor.matmul(
                    yp,
                    lhsT=atp[j][:, ni * 128:(ni + 1) * 128],
                    rhs=pwT[j],
                    start=(j == 0), stop=(j == heads // 2 - 1),
                )
            yout = ypool.tile([128, dim], f32, tag="yout", name="yout")
            nc.vector.tensor_add(yout, yp, pb_bc)
            eng = nc.sync if ni % 2 == 0 else nc.scalar
            eng.dma_start(y_d[ni * 128:(ni + 1) * 128, :], yout)

        es_bc = es.enter_context(ExitStack())
        sps = es_bc.enter_context(tc.tile_pool(name="sps", bufs=2, space="PSUM"))
        expp = es_bc.enter_context(tc.tile_pool(name="expp", bufs=1))
        zstp = es_bc.enter_context(tc.tile_pool(name="zst", bufs=2))
        zdp = es_bc.enter_context(tc.tile_pool(name="zdram", bufs=2, space="DRAM"))
        rbp = es_bc.enter_context(tc.tile_pool(name="rbp", bufs=2))
        avps = es_bc.enter_context(tc.tile_pool(name="avp", bufs=2, space="PSUM"))
        xtp = tc.alloc_tile_pool(name="xt", bufs=1)
        xTt = xtp.tile([128, CC, n_tok], bf16, tag="xTt", name="xTt")
        xT = [xTt[:, cc, :] for cc in range(CC)]
        bps = tc.alloc_tile_pool(name="bps", bufs=2, space="PSUM")
        xinp = tc.alloc_tile_pool(name="xin", bufs=3)

        def emit_warmup():
            # tiny exp on a const tile: pulls in the ACT exp table (~2.7us)
            # while the x DMAs are still in flight
            nc.scalar.activation(junk_e, ident[:, 0:8], AF.Exp,
                                 scale=EXP2_ACT_SCALE)
            for _ in range(12):
                jp = bps.tile([128, 128], f32, tag="bps", name="jp")
                nc.tensor.matmul(jp, lhsT=junk_sb, rhs=junk_sb,
                                 start=True, stop=True)

        def emit_xchunk(ni):
            xin = xinp.tile([128, dim], bf16, tag="xin", name="xin")
            eng = (nc.sync, nc.scalar)[ni % 2]
            eng.dma_start(xin, x_d[ni * 128:(ni + 1) * 128, :])
            pt = bps.tile([128, CC, 128], bf16, tag="bps", name="pt")
            for cc in range(CC):
                nc.tensor.transpose(
                    pt[:, cc, :], xin[:, cc * 128:(cc + 1) * 128], ident)
            nc.vector.tensor_copy(xTt[:, :, ni * 128:(ni + 1) * 128], pt)

        def emit_qkT_chunk(jc, q4):
            ps = bps.tile([128, 512], f32, tag="bps", name="qkps")
            for cc in range(CC):
                nc.tensor.matmul(
                    ps,
                    lhsT=qkwT[cc][:, jc * 128:(jc + 1) * 128],
                    rhs=xT[cc][:, q4 * 512:(q4 + 1) * 512],
                    start=(cc == 0), stop=(cc == CC - 1),
                )
            nc.vector.tensor_scalar_add(
                qkT[jc][:, q4 * 512:(q4 + 1) * 512], ps, qkb[jc])

        def emit_v_chunk(ni):
            ps = avps.tile([128, dim], f32, tag="av", name="vps")
            for cc in range(CC):
                nc.tensor.matmul(
                    ps,
                    lhsT=xT[cc][:, ni * 128:(ni + 1) * 128],
                    rhs=vwT[cc],
                    start=(cc == 0), stop=(cc == CC - 1),
                )
            nc.vector.tensor_tensor(
                vp[ni][:, :, 0:64],
                ps[:, 0:dim].rearrange("p (h d) -> p h d", h=heads),
                vb_bc.rearrange("p (h d) -> p h d", h=heads),
                ALU.add,
            )
            nc.gpsimd.memset(vp[ni][:, :, 64:65], 1.0)

        def emit_scores_kc(h, half, kc, ets, pool=None, pfx="e"):
            q0 = half * HALF
            sp = sps.tile([128, 1024], f32, tag="sp", name="sp")
            for qs in range(2):
                nc.tensor.matmul(
                    sp[:, qs * 512:(qs + 1) * 512],
                    lhsT=qk_slice(dim + h * 64, kc * 128, 128),
                    rhs=qk_slice(h * 64, q0 + qs * 512, 512),
                    start=True, stop=True,
                )
            et = (pool or expp).tile([128, 1024], bf16,
                                     tag=f"{pfx}{kc}", name=f"{pfx}{kc}")
            nc.scalar.activation(et, sp, AF.Exp, scale=EXP2_ACT_SCALE)
            ets.append(et)

        def emit_scores_kc_dve(h, half, kc, ets, pool=None, pfx="e"):
            q0 = half * HALF
            sp = sps.tile([128, 1024], f32, tag="sp", name="sp")
            for qs in range(2):
                nc.tensor.matmul(
                    sp[:, qs * 512:(qs + 1) * 512],
                    lhsT=k65[h][:, k65slot[h][kc] * 128:(k65slot[h][kc] + 1) * 128],
                    rhs=q65[h][:, q0 + qs * 512:q0 + (qs + 1) * 512],
                    start=True, stop=True,
                )
            et = (pool or expp).tile([128, 1024], bf16,
                                     tag=f"{pfx}{kc}", name=f"{pfx}{kc}")
            nc.vector._custom_dve(EXP2, out=et[:, :].bitcast(i16), in0=sp,
                                  in1=cft, s0=EXP2_MAGIC, s1=EXP2_C_LIN,
                                  imm2=EXP2_C_QUAD)
            ets.append(et)

        def emit_qk65_fill(h):
            """q65/k65 for head h from the finished qkT rows (sb->sb DMA)."""
            po = (h % 2) * 64
            nc.sync.dma_start(q65[h][0:64, :], qkT[h // 2][po:po + 64, :])
            for kc, sl in k65slot[h].items():
                nc.sync.dma_start(
                    k65[h][0:64, sl * 128:(sl + 1) * 128],
                    qkT[JC // 2 + h // 2][po:po + 64, kc * 128:(kc + 1) * 128])

        def emit_av(h, half, ets, zstage):
            # kc outer so both qs matmuls share one LDWEIGHTS of vp[kc]
            avs = [avps.tile([65, 512], f32, tag="av", name=f"av{qs}")
                   for qs in range(2)]
            for kc in range(NCH):
                for qs in range(2):
                    nc.tensor.matmul(
                        avs[qs],
                        lhsT=vp[kc][:, h, :],
                        rhs=ets[kc][:, qs * 512:(qs + 1) * 512],
                        start=(kc == 0), stop=(kc == NCH - 1),
                    )
            for qs in range(2):
                qc = half * 2 + qs
                nc.vector.tensor_copy(
                    attnT[h][:, qc * 512:(qc + 1) * 512], avs[qs][0:64, :])
                nc.vector.tensor_copy(
                    zstage[64:65, qc * 512:(qc + 1) * 512], avs[qs][64:65, :])

        def emit_pe_warm(dep_ap, nf):
            # tiny matmul reading `dep_ap` -- keeps the PE HAM window warm
            # across the final norm chain so proj doesn't start down-clocked
            p = dep_ap.partition_size()
            jt = avps.tile([1, nf], f32, tag="av", name="junkt")
            nc.tensor.matmul(jt, lhsT=dep_ap[0:p, 0:1], rhs=dep_ap[0:p, 0:nf],
                             start=True, stop=True)

        def emit_norm_half(h, zstage, half, warm=False):
            # 1/Z for this half's columns, bounce through DRAM to broadcast
            # across partitions, then normalize attnT in place. The
            # reciprocal runs on a [128, HALF/128] reshape (a [1, n] row
            # would be ~16us).
            c0 = half * HALF
            zcol = zstp.tile([128, HALF // 128], f32, tag="zcol", name="zcol")
            nc.sync.dma_start(zcol, zstage[64:65, c0:c0 + HALF])
            nc.vector.reciprocal(zcol, zcol)
            zd = zdp.tile([1, HALF], f32, tag="zd", name="zd")
            nc.sync.dma_start(zd, zcol)
            rb = rbp.tile([128, HALF], f32, tag="rb", name="rb")
            if warm:
                emit_pe_warm(zcol, HALF // 128)
            nc.gpsimd.dma_start(
                out=rb,
                in_=bass.AP(tensor=zd.tensor, offset=zd.offset,
                            ap=[[0, 128], zd.ap[-1]]),
            )
            if warm:
                emit_pe_warm(rb, 512)
                emit_pe_warm(rb[:, 512:], 512)
            po = (h % 2) * 64
            for qs in range(HALF // 512):
                nc.gpsimd.tensor_tensor(
                    attnT[h][:, c0 + qs * 512:c0 + (qs + 1) * 512],
                    attnT[h][:, c0 + qs * 512:c0 + (qs + 1) * 512],
                    rb[po:po + 64, qs * 512:(qs + 1) * 512], ALU.mult)

        def emit_head(h, extras=()):
            """One head; `extras` are (kc_index, closure) emitted inside the
            half-0 score loop to soak spare PE cycles under the exp stream."""
            extras = dict(extras)
            zstage = zstp.tile([65, n_tok], f32, tag="zst", name="zst")
            last = h == heads - 1
            for half in range(2):
                ets = []
                for kc in range(NCH):
                    if kc in dve_set[h]:
                        emit_scores_kc_dve(h, half, kc, ets)
                    else:
                        emit_scores_kc(h, half, kc, ets)
                    fn = extras.pop((half, kc), None)
                    if fn is not None:
                        fn()
                emit_av(h, half, ets, zstage)
                emit_norm_half(h, zstage, half, warm=(last and half == 1))
            for fn in extras.values():
                fn()

        if NQ4 == 4 and heads == 6:
            # Pipelined startup: per 512-column group load/transpose x,
            # produce that group's qkT columns for head-0's q/k row-chunks,
            # and start head-0 scores as soon as their inputs exist. v' and
            # the remaining qkT chunks ride in PE slack under the exp stream.
            zstage0 = zstp.tile([65, n_tok], f32, tag="zst", name="zstage0")
            emit_warmup()
            ets0 = []

            def emit_scores_512(kc, qs):
                sp = sps.tile([128, 512], f32, tag="sp", name="sp")
                nc.tensor.matmul(
                    sp,
                    lhsT=qk_slice(dim, kc * 128, 128),
                    rhs=qk_slice(0, qs * 512, 512),
                    start=True, stop=True,
                )
                if qs == 0:
                    et = expp.tile([128, 1024], bf16, tag=f"e{kc}", name=f"e{kc}")
                    ets0.append(et)
                nc.scalar.activation(ets0[kc][:, qs * 512:(qs + 1) * 512],
                                     sp, AF.Exp, scale=EXP2_ACT_SCALE)

            for q4 in range(4):
                for ni in range(4 * q4, 4 * q4 + 4):
                    emit_xchunk(ni)
                emit_qkT_chunk(0, q4)
                emit_qkT_chunk(JC // 2, q4)
                if q4 == 0:
                    for kc in range(0, 4):
                        emit_scores_512(kc, 0)
                elif q4 == 1:
                    for kc in range(0, 4):
                        emit_scores_512(kc, 1)
                    for kc in range(4, 8):
                        emit_scores_kc(0, 0, kc, ets0)
                    for ni in range(0, 4):
                        emit_v_chunk(ni)
                elif q4 == 2:
                    for kc in range(8, 12):
                        emit_scores_kc(0, 0, kc, ets0)
                    for ni in range(4, 10):
                        emit_v_chunk(ni)
                elif q4 == 3:
                    for kc in range(12, 16):
                        emit_scores_kc(0, 0, kc, ets0)
                    for ni in range(10, 16):
                        emit_v_chunk(ni)
            # h0 half1 runs from its own short-lived exp tile set so its
            # exp stream does not wait on AV(0,0)'s reads of the half-0 set
            expp2 = tc.alloc_tile_pool(name="expp2", bufs=1)
            emit_qk65_fill(0)
            emit_qk65_fill(1)
            ets1 = []
            for kc in range(0, 9):
                emit_scores_kc(0, 1, kc, ets1, pool=expp2, pfx="f")
            emit_av(0, 0, ets0, zstage0)
            emit_norm_half(0, zstage0, 0)
            for kc in range(9, NCH):
                if kc in dve_set[0]:
                    emit_scores_kc_dve(0, 1, kc, ets1, pool=expp2, pfx="f")
                else:
                    emit_scores_kc(0, 1, kc, ets1, pool=expp2, pfx="f")
            emit_av(0, 1, ets1, zstage0)
            emit_norm_half(0, zstage0, 1)
            expp2.release()
            # remaining qkT chunks: one j-chunk (4 q4-groups) per half
            jc_sched = {
                1: (((0, 1), lambda: emit_qkT_chunk(4, 0)),
                    ((0, 5), lambda: emit_qkT_chunk(4, 1)),
                    ((0, 9), lambda: emit_qkT_chunk(4, 2)),
                    ((0, 13), lambda: emit_qkT_chunk(4, 3)),
                    ((0, 3), lambda: emit_qkT_chunk(1, 0)),
                    ((0, 11), lambda: emit_qkT_chunk(1, 1)),
                    ((1, 3), lambda: emit_qkT_chunk(1, 2)),
                    ((1, 11), lambda: emit_qkT_chunk(1, 3)),
                    ((1, 1), lambda: emit_qkT_chunk(2, 0)),
                    ((1, 5), lambda: emit_qkT_chunk(2, 1)),
                    ((1, 9), lambda: emit_qkT_chunk(2, 2)),
                    ((1, 13), lambda: emit_qkT_chunk(2, 3))),
                2: (((0, 1), lambda: emit_qkT_chunk(5, 0)),
                    ((0, 5), lambda: emit_qkT_chunk(5, 1)),
                    ((0, 9), lambda: emit_qkT_chunk(5, 2)),
                    ((0, 13), lambda: emit_qkT_chunk(5, 3))),
            }
            emit_head(1, jc_sched[1])
            emit_qk65_fill(2)
            emit_qk65_fill(3)
            emit_head(2, jc_sched[2])
            emit_qk65_fill(4)
            emit_qk65_fill(5)
            xinp.release()
            bps.release()
            xtp.release()
            first_rest = 3
        else:
            # simple sequential fallback (small configs / sim)
            for ni in range(NCH):
                emit_xchunk(ni)
            for jc in range(JC):
                for q4 in range(NQ4):
                    emit_qkT_chunk(jc, q4)
            for ni in range(NCH):
                emit_v_chunk(ni)
            xinp.release()
            bps.release()
            xtp.release()
            first_rest = 0
        for h in range(first_rest, heads):
            if first_rest == 0:
                emit_qk65_fill(h)
            emit_head(h)
        drop_proj_pools()
        es_bc.close()  # release attention pools

        # ---- phase D: remaining projection chunks ----
        make_proj_pools()
        for ni in range(NCH):
            emit_proj_chunk(ni)
        drop_proj_pools()

    nc.compile()
    return nc


def make_in_maps(x, qkv_w, qkv_b, proj_w, proj_b, n_cores=N_CORES):
    """Host-side shard prep: per-core input dicts (weights host-transposed)."""
    x = np.asarray(x, dtype=np.float32)
    qkv_w = np.asarray(qkv_w, dtype=np.float32)
    qkv_b = np.asarray(qkv_b, dtype=np.float32)
    proj_w = np.asarray(proj_w, dtype=np.float32)
    proj_b = np.asarray(proj_b, dtype=np.float32)
    dim = x.shape[-1]
    import ml_dtypes
    bf16 = ml_dtypes.bfloat16
    qscale = 128.0 * (DK ** -0.5) * 1.4426950408889634
    qkw = qkv_w[:2 * dim].copy()
    qkw[:dim] *= qscale
    qkb = qkv_b[:2 * dim].copy()
    qkb[:dim] *= qscale
    # proj weights in head-pair row-stacked layout [pw_{2j}; pw_{2j+1}]
    pwp = proj_w.T  # [dim(in), dim(out)] rows = attn dims
    shared = {
        "qkw_t": np.ascontiguousarray(qkw.T.astype(bf16)),
        "vw_t": np.ascontiguousarray(qkv_w[2 * dim:3 * dim].T.astype(bf16)),
        "pw_t": np.ascontiguousarray(pwp.astype(bf16)),
        "qk_b": np.ascontiguousarray(qkb),
        "v_b": np.ascontiguousarray(qkv_b[2 * dim:3 * dim]),
        "p_b": np.ascontiguousarray(proj_b),
    }
    return [
        {"x_b": np.ascontiguousarray(x[i].astype(bf16)), **shared}
        for i in range(x.shape[0])
    ]


def run_on_hw(nc, in_maps, trace=False, trace_cores=None):
    from concourse import bass_utils
    return bass_utils.run_bass_kernel_spmd(
        nc, in_maps, core_ids=list(range(len(in_maps))),
        trace=trace, trace_cores=trace_cores,
    )


def kernel(x, qkv_w, qkv_b, proj_w, proj_b):
    key = (N_TOK, DIM, HEADS)
    if key not in _module_cache:
        _module_cache[key] = build_module(*key)
    nc = _module_cache[key]
    in_maps = make_in_maps(x, qkv_w, qkv_b, proj_w, proj_b)
    res = run_on_hw(nc, in_maps)
    y = np.stack([res.results[i]["y_b"] for i in range(len(in_maps))])
    return y.astype(np.float32)


if __name__ == "__main__":
    import reference
    inputs = reference.setup_inputs()
    out = kernel(**{k: np.asarray(v) for k, v in inputs.items()})
    print("out", out.shape, out.dtype)

